# revision 1
# baseline (speedup 1.0000x reference)
"""Trainium2 Bass kernel for nn_Attention_86655260164689.

Computation (per batch b of 16):
  qe = causal_conv1d(q[b], wq); ke = causal_conv1d(v[b], wk); ve = causal_conv1d(k[b], wv)
  scores = qe^T ke / sqrt(8)      [S, S], S=2048
  attn   = softmax(scores, -1)
  out    = w_out @ (ve @ attn^T) + b_out   -> y[b] = [8, S]

Sharding: data-parallel over batch, 2 batches per NeuronCore on 8 cores.

Device strategy per batch:
  - convs for q/k/v fused into one matmul: im2col [60, S] x wblk [60, 24]
    (wv pre-multiplied by w_out on host; ke/ve input swap from the reference
    is baked into wblk's row layout).
  - scores computed transposed: scoresT[t, s] = sum_c ke[c,t] qe[c,s] via
    K=8 matmuls (lhsT = ke chunk, rhs = qe), PSUM [128t, 1024s] tiles.
  - exp on ScalarE (PSUM->SBUF), scale 1/sqrt(8) folded in. No max
    subtraction needed: |scores|/sqrt(8) stays far below f32 exp overflow.
  - attn @ ve^T and the softmax denominator in one PSUM accumulation:
    lhsT = [ve^T | ones] [128t, 9], rhs = expT chunk -> av[9, s] where
    row 8 is the denominator. ve^T chunks come straight from swapped-operand
    conv matmuls (im2col^T @ wv2), so ve never needs a PSUM->SBUF copy or a
    PE transpose.
  - normalize in [c, s] layout: denominator row -> DRAM -> partition-
    broadcast DMA -> reciprocal -> multiply -> per-partition bias add
    (tail quarters use a PE-transpose path instead, avoiding the DRAM
    round-trip latency on the kernel's critical exit path).
  - matmul operands are float32r (tf32-class) for full-rate PE throughput;
    accumulation stays fp32 in PSUM. Phase A (conv) and
    phase C (normalization) are interleaved into the score/exp/av chunk
    pipeline as emission-order insertions so ScalarE (the bottleneck:
    ~8.4M exp evaluations/core) stays busy across batch boundaries.
"""

import sys

sys.path.insert(0, "/opt/trn_rl_repo")

import numpy as np

import concourse.bass as bass
import concourse.mybir as mybir
import concourse.tile as tile
from concourse.bass_utils import run_bass_kernel_spmd
from concourse.masks import make_identity

F32 = mybir.dt.float32
F32R = mybir.dt.float32r
EXPF = mybir.ActivationFunctionType.Exp

B, C_IN, C_OUT, K, S = 16, 4, 8, 5, 2048
NCORES = 8
BPC = B // NCORES          # batches per core
PAD = K - 1                # left reflect pad
IM2_P = C_IN * 3 * K       # 60 im2col partitions
EMB_P = 72                 # conv out rows: qe@0, ke@32, ve@64 (32-aligned for DVE reads)
SCALE = 1.0 / np.sqrt(float(C_OUT))
NT = S // 128              # 16 t-chunks
NHALF = 2
SH = S // NHALF            # 1024 s columns per half


def _split_waits(nc, limit=1):
    """Workaround: tile's tail drain carries more sem waits than this
    walrus build can encode on one instruction; hoist extras onto NoOps."""
    f = nc.m.functions[0]
    for bb in f.blocks:
        insts = list(bb.instructions)
        changed = False
        new = []
        for inst in insts:
            si = inst.sync_info
            if si is not None and si.on_wait is not None and len(si.on_wait) > limit:
                waits = list(si.on_wait)
                for w in waits[limit:]:
                    nop = mybir.InstNoOp(
                        name=nc.get_next_instruction_name(),
                        engine=inst.engine,
                        sync_info=mybir.SyncInfo(on_wait=[w], on_update=[]),
                    )
                    nc.register_instruction(nop)
                    new.append(nop)
                inst.sync_info = mybir.SyncInfo(
                    on_wait=waits[:limit], on_update=list(si.on_update or [])
                )
                changed = True
            new.append(inst)
        if changed:
            bb.instructions = new


def _trim_exit_barrier(nc):
    """Drop the second all-engine barrier after the tail semaphore clear.
    NRT waits for every engine stream to finish before returning, so the
    post-clear re-sync only adds exit latency."""
    f = nc.m.functions[0]
    bb = f.blocks[-1]
    insts = list(bb.instructions)
    last_isa = None
    for i, inst in enumerate(insts):
        if type(inst).__name__ == "InstISA" and str(inst.engine).endswith("Pool"):
            last_isa = i
    if last_isa is None:
        return
    tail = insts[last_isa + 1 :]
    if tail and all(
        type(t).__name__ in ("InstDrain", "InstEventSemaphore", "InstNoOp")
        for t in tail
    ):
        bb.instructions = insts[: last_isa + 1]


def _dram_bc(ap, nparts):
    """Partition-broadcast view of a [1, N] DRAM AP."""
    return bass.AP(tensor=ap.tensor, offset=ap.offset, ap=[[0, nparts]] + list(ap.ap[1:]))


def _build():
    nc = bass.Bass()
    im2_d = nc.declare_dram_parameter("im2", [BPC, IM2_P, S], F32R, isOutput=False)
    wblk_d = nc.declare_dram_parameter("wblk", [IM2_P, EMB_P], F32R, isOutput=False)
    bias_d = nc.declare_dram_parameter("bias", [C_OUT, 1], F32, isOutput=False)
    y_d = nc.declare_dram_parameter("y", [BPC, C_OUT, S], F32, isOutput=True)
    scr_d = nc.dram_tensor("scr", [BPC, NHALF, 2, 512], F32)

    with tile.TileContext(nc) as tc:
        with (
            tc.tile_pool(name="singles", bufs=1) as singles,
            tc.tile_pool(name="sbuf", bufs=2) as sb,
            tc.tile_pool(name="expp", bufs=4) as expp,
            tc.tile_pool(name="scpool", bufs=3, space="PSUM") as scps,
            tc.tile_pool(name="avpool", bufs=2, space="PSUM") as avps,
        ):
            ident = singles.tile([128, 128], F32)
            wblk = singles.tile([IM2_P, EMB_P], F32R)
            bias = singles.tile([C_OUT, 1], F32)
            im2a = sb.tile([IM2_P, S], F32R, tag="im2")
            im2b = sb.tile([IM2_P, S], F32R, tag="im2")
            im2s = [im2a, im2b]
            # warm the ACT exp table before anything else queues on ScalarE
            warm = singles.tile([128, 16], F32)
            nc.gpsimd.memset(warm, 0.0)
            nc.scalar.activation(out=warm, in_=warm, func=EXPF, scale=1.0)
            nc.sync.dma_start(out=im2a[:, 0:1024], in_=im2_d[0][:, 0:1024])
            nc.scalar.dma_start(out=wblk, in_=wblk_d[:, :])
            nc.sync.dma_start(out=im2a[:, 1024:2048], in_=im2_d[0][:, 1024:2048])
            nc.scalar.dma_start(out=bias, in_=bias_d[:, :])
            nc.sync.dma_start(out=im2b, in_=im2_d[1])
            make_identity(nc, ident)
            # warm the PE clock gate (HAM) during the input-DMA window so the
            # first conv/score matmuls run at full rate
            wps = scps.tile([128, 128], F32, tag="sc", name="warmps")
            for _wi in range(3):
                nc.tensor.matmul(wps, lhsT=ident, rhs=ident, start=True, stop=True)

            # deferred post-processing closures, drained at spread points
            # inside later chunk loops so DVE work never clumps at
            # batch/half boundaries
            pending = []

            def emit_conv_half(b, h, qe, ke):
                h0 = h * 1024
                emb = scps.tile([EMB_P, 1024], F32, tag="sc", name=f"emb{b}{h}")
                for ns in range(2):
                    nc.tensor.matmul(
                        emb[:, ns * 512 : (ns + 1) * 512],
                        lhsT=wblk,
                        rhs=im2s[b][:, h0 + ns * 512 : h0 + (ns + 1) * 512],
                        start=True,
                        stop=True,
                    )
                if h == 0:
                    # ke on DVE; qe on the (idle-at-batch-start) ScalarE so the
                    # first score matmuls start early
                    nc.vector.tensor_copy(out=ke[:, 0:128], in_=emb[32:40, 0:128])
                    nc.scalar.copy(out=qe[:, 0:512], in_=emb[0:8, 0:512])
                    nc.scalar.copy(out=qe[:, 512:1024], in_=emb[0:8, 512:1024])
                    nc.vector.tensor_copy(out=ke[:, 128:1024], in_=emb[32:40, 128:1024])
                else:
                    # qe upper half is only needed in s-half 1; copy it last
                    nc.vector.tensor_copy(out=ke[:, h0 : h0 + 1024], in_=emb[32:40, :])
                    nc.vector.tensor_copy(out=qe[:, h0 : h0 + 1024], in_=emb[0:8, :])

            def emit_vet_group(b, tg, veaug):
                # ve^T chunks straight from the conv: [128s, 8] = im2^T @ wv2.
                # No PSUM->SBUF ve copy and no PE transpose chain needed.
                vt = scps.tile([128, 4, C_OUT], F32, tag="sc", name=f"vt{b}{tg}")
                for ti in range(4):
                    t = tg * 4 + ti
                    nc.tensor.matmul(
                        vt[:, ti, :],
                        lhsT=im2s[b][:, t * 128 : (t + 1) * 128],
                        rhs=wblk[:, 64:72],
                        start=True,
                        stop=True,
                    )
                nc.vector.tensor_copy(
                    out=veaug[:, tg * 4 : (tg + 1) * 4, 0:C_OUT], in_=vt
                )

            def make_quarter(b, sh, jq, av_t, outT, tp_path=False):
                s0 = sh * SH

                def emit():
                    q0 = jq * 512
                    av_sb = sb.tile(
                        [C_OUT + 1, 512], F32, tag="av_sb", name=f"avsb{b}{sh}{jq}"
                    )
                    if tp_path and jq == 1:
                        # tail: second quarter's PSUM->SBUF copy on the now-idle
                        # ScalarE so both quarters' chains run concurrently
                        nc.scalar.copy(out=av_sb, in_=av_t)
                    else:
                        nc.vector.tensor_copy(out=av_sb, in_=av_t)
                    if tp_path:
                        # tail-only: transpose-path normalization (no DRAM
                        # round-trip, PSUM slots are idle here)
                        ot = scps.tile(
                            [C_OUT, 512], F32, tag="sc", name=f"ot{b}{sh}{jq}"
                        )
                        for j in range(4):
                            tp = scps.tile(
                                [128, C_OUT + 1], F32, tag="sc", name=f"tp{b}{sh}{jq}{j}"
                            )
                            nc.tensor.transpose(
                                tp,
                                in_=av_sb[:, j * 128 : (j + 1) * 128],
                                identity=ident[0 : C_OUT + 1, 0 : C_OUT + 1],
                            )
                            rcp = sb.tile(
                                [128, 1], F32, tag="rcp", name=f"rcp{b}{sh}{jq}{j}"
                            )
                            nc.vector.reciprocal(out=rcp, in_=tp[:, C_OUT : C_OUT + 1])
                            at = sb.tile(
                                [128, C_OUT], F32, tag="at", name=f"at{b}{sh}{jq}{j}"
                            )
                            nc.vector.tensor_scalar_mul(
                                out=at, in0=tp[:, 0:C_OUT], scalar1=rcp
                            )
                            nc.tensor.transpose(
                                ot[:, j * 128 : (j + 1) * 128], in_=at, identity=ident
                            )
                        nc.vector.tensor_scalar_add(
                            out=outT[:, s0 + q0 : s0 + q0 + 512], in0=ot, scalar1=bias
                        )
                        nc.sync.dma_start(
                            out=y_d[b, :, s0 + q0 : s0 + q0 + 512],
                            in_=outT[:, s0 + q0 : s0 + q0 + 512],
                        )
                    else:
                        # denominator -> DRAM -> broadcast across 8 partitions
                        scr = scr_d[b, sh, jq][None, :]
                        nc.sync.dma_start(out=scr, in_=av_sb[C_OUT : C_OUT + 1, :])
                        bc = sb.tile([C_OUT, 512], F32, tag="bc", name=f"bc{b}{sh}{jq}")
                        nc.sync.dma_start(out=bc, in_=_dram_bc(scr, C_OUT))
                        nc.vector.reciprocal(out=bc, in_=bc)
                        nc.vector.tensor_mul(
                            out=outT[:, s0 + q0 : s0 + q0 + 512],
                            in0=av_sb[0:C_OUT, :],
                            in1=bc,
                        )
                        nc.vector.tensor_scalar_add(
                            out=outT[:, s0 + q0 : s0 + q0 + 512],
                            in0=outT[:, s0 + q0 : s0 + q0 + 512],
                            scalar1=bias,
                        )

                return emit

            def make_store(b, sh, outT, skip=False):
                s0 = sh * SH

                def emit():
                    if not skip:
                        nc.sync.dma_start(
                            out=y_d[b, :, s0 : s0 + SH], in_=outT[:, s0 : s0 + SH]
                        )

                return emit

            DRAIN_AT = (3, 7, 10, 13)
            state = {}
            for b in range(BPC):
                qe = sb.tile([C_OUT, S], F32R, tag="qe")
                ke = sb.tile([C_OUT, S], F32R, tag="ke")
                veaug = sb.tile([128, NT, C_OUT + 1], F32R, tag="veaug")
                vones = sb.tile([128, NT, C_OUT + 1], F32, tag="vones")
                outT = sb.tile([C_OUT, S], F32, tag="outT")
                state[b] = (qe, ke, veaug, outT)
                for sh in range(NHALF):
                    if sh == 0:
                        emit_conv_half(b, 0, qe, ke)
                        nc.vector.memset(vones, 1.0)
                        nc.vector.tensor_copy(out=veaug, in_=vones)
                    s0 = sh * SH
                    av0 = avps.tile([C_OUT + 1, 512], F32, tag="av")
                    av1 = avps.tile([C_OUT + 1, 512], F32, tag="av")
                    avq = [av0, av1]
                    ex_prev = None
                    for t in range(NT + 1):
                        ex = None
                        if t < NT:
                            sc = scps.tile([128, SH], F32, tag="sc")
                            for ns in range(2):
                                nc.tensor.matmul(
                                    sc[:, ns * 512 : (ns + 1) * 512],
                                    lhsT=ke[:, t * 128 : (t + 1) * 128],
                                    rhs=qe[:, s0 + ns * 512 : s0 + (ns + 1) * 512],
                                    start=True,
                                    stop=True,
                                )
                            ex = expp.tile([128, SH], F32R)
                            nc.scalar.activation(out=ex, in_=sc, func=EXPF, scale=SCALE)
                        if t >= 1:
                            for ns in range(2):
                                nc.tensor.matmul(
                                    avq[ns][:, :],
                                    lhsT=veaug[:, t - 1, :],
                                    rhs=ex_prev[:, ns * 512 : (ns + 1) * 512],
                                    start=(t - 1 == 0),
                                    stop=(t - 1 == NT - 1),
                                )
                        ex_prev = ex
                        # phase-A insertions woven into the first half
                        if sh == 0:
                            if t == 0:
                                emit_vet_group(b, 0, veaug)
                            elif t == 2:
                                emit_vet_group(b, 1, veaug)
                            elif t == 4:
                                emit_conv_half(b, 1, qe, ke)
                            elif t in (6, 7):
                                emit_vet_group(b, t - 4, veaug)
                        if t in DRAIN_AT and pending:
                            pending.pop(0)()
                    last = b == BPC - 1 and sh == NHALF - 1
                    pending.append(make_quarter(b, sh, 0, av0, outT, tp_path=last))
                    pending.append(make_quarter(b, sh, 1, av1, outT, tp_path=last))
                    pending.append(make_store(b, sh, outT, skip=last))
            for fn in pending:
                fn()

    _split_waits(nc)
    _trim_exit_barrier(nc)
    return nc


_NC = None


def _get_nc():
    global _NC
    if _NC is None:
        _NC = _build()
    return _NC


def _prep_weights(wq, wk, wv, w_out):
    wq = np.asarray(wq, np.float32)
    wk = np.asarray(wk, np.float32)
    wv = np.asarray(wv, np.float32)
    w_out = np.asarray(w_out, np.float32)
    wv2 = np.einsum("oc,cik->oik", w_out, wv).astype(np.float32)
    wblk = np.zeros((IM2_P, EMB_P), np.float32)
    for kk in range(K):
        for ci in range(C_IN):
            wblk[kk * 12 + ci, 0:8] = wq[:, ci, kk]          # qe from q
            wblk[kk * 12 + 8 + ci, 32:40] = wk[:, ci, kk]    # ke from v (source swap)
            wblk[kk * 12 + 4 + ci, 64:72] = wv2[:, ci, kk]   # w_out @ ve from k
    return wblk


def _im2col(q, k, v):
    """Host-side layout staging: reflect-pad and stack shifted views so the
    on-device conv is a single [60, 24] matmul. Row r = kk*12 + j maps to
    input j (0-3: q, 4-7: k, 8-11: v) at tap kk."""
    xq = np.pad(q, ((0, 0), (0, 0), (PAD, 0)), mode="reflect")
    xk = np.pad(k, ((0, 0), (0, 0), (PAD, 0)), mode="reflect")
    xv = np.pad(v, ((0, 0), (0, 0), (PAD, 0)), mode="reflect")
    im2 = np.empty((q.shape[0], IM2_P, S), np.float32)
    for kk in range(K):
        im2[:, kk * 12 + 0 : kk * 12 + 4] = xq[:, :, kk : kk + S]
        im2[:, kk * 12 + 4 : kk * 12 + 8] = xk[:, :, kk : kk + S]
        im2[:, kk * 12 + 8 : kk * 12 + 12] = xv[:, :, kk : kk + S]
    return im2


def run(q, k, v, wq, wk, wv, w_out, b_out, trace=False):
    nc = _get_nc()
    q = np.asarray(q, np.float32)
    k = np.asarray(k, np.float32)
    v = np.asarray(v, np.float32)
    im2 = _im2col(q, k, v)
    wblk = _prep_weights(wq, wk, wv, w_out)
    bias = np.asarray(b_out, np.float32).reshape(C_OUT, 1)
    in_maps = []
    for c in range(NCORES):
        sl = slice(c * BPC, (c + 1) * BPC)
        in_maps.append(
            {
                "im2": np.ascontiguousarray(im2[sl]),
                "wblk": wblk,
                "bias": bias,
            }
        )
    res = run_bass_kernel_spmd(nc, in_maps, core_ids=list(range(NCORES)), trace=trace)
    y = np.concatenate([res.results[c]["y"] for c in range(NCORES)], axis=0)
    return y, res


def kernel(q, k, v, wq, wk, wv, w_out, b_out):
    y, _ = run(q, k, v, wq, wk, wv, w_out, b_out, trace=False)
    return y

